# revision 1
# baseline (speedup 1.0000x reference)
"""Trainium2 Bass kernel: batched 3x3 Lorenz-Jacobian Taylor matrix exponential.

Per element (u, x1, x2) = x[n, :]:
    A = dt * [[-10, 10, 0], [28, -1, -u], [0, u, -8/3]]
    F = sum_{j=0..5} A^j / j!          (truncated Taylor expm)
    y = F @ x

Structure exploited: every entry of F is an even or odd polynomial in u, and
F[2][1] = -F[1][2].  With v = u^2, w = v^2:
    F00 = a0 + a2 v
    F01 = b0 + b2 v (+ b4 w: dropped, contribution < 1.4e-6)
    F02 = u * (c1 + c3 v)
    F10 = d0 + d2 v (+ d4 w: dropped, < 2.9e-6)
    F11 = e0 + e2 v (+ e4 w: dropped, < 1.9e-5)
    F12 = u * (f1 + f3 v) (+ f5 w u: dropped, < 8e-7)
    F20 = u * (g1 + g3 v)
    F21 = -F12
    F22 = h0 + h2 v (+ h4 w: dropped, < 1.9e-5)

    y0 = ah*u + bh*x1 + ch*Q          Q = u*x2
    y1 = dh*u + eh*x1 + fh*Q
    y2 = gh*v + hh*x2 - fh*P          P = u*x1

(validated vs float64 reference: max scale-relative error 1.4e-6 in fp32)

Layout: batch sharded 8 ways; per core T tiles of [128 partitions, 3E floats]
(each partition row = E consecutive elements, x0 x1 x2 interleaved).
Component views are stride-3 APs; products/entries are dense [128, E] tiles.
Work split across ACT (squares + affine), DVE and GPSIMD (products, dots).
"""

import numpy as np
from contextlib import ExitStack

import concourse.bass as bass
import concourse.tile as tile
import concourse.mybir as mybir
from concourse.bass_utils import run_bass_kernel_spmd

DT = 0.02

# float64-exact Taylor coefficients of F entries (poly in v = u^2)
A0, A2 = 0.8679133685333335, -1.6824888888888892e-06
B0, B2 = 0.1827780802666667, -1.254811851851852e-05
C1, C3 = -0.0018440311802469136, 6.204444444444445e-08
D0, D2 = 0.5117786247466667, -3.513473185185185e-05
E0, E2, E4 = 1.0324136407733333, -0.00019737891358024691, 6.4444444444444465e-09
F1, F3 = -0.019630097558847738, 1.3003111111111115e-06
G1, G3 = 0.005163287304691359, -1.737244444444445e-07
H0, H2, H4 = 0.9480639384616735, -0.00019347448395061728, 6.400000000000002e-09

NCORES = 8
E_DEF = 489      # elements per partition row per tile
T_DEF = 4        # tiles per core
B_IN = 2_000_000

FP32 = mybir.dt.float32
MULT = mybir.AluOpType.mult
ADD = mybir.AluOpType.add
SUB = mybir.AluOpType.subtract
IDENT = mybir.ActivationFunctionType.Identity


def make_pools(tc, ctx):
    return {
        "xp": ctx.enter_context(tc.tile_pool(name="xp", bufs=3)),
        "pp": ctx.enter_context(tc.tile_pool(name="pp", bufs=3)),
        "ep": ctx.enter_context(tc.tile_pool(name="ep", bufs=3)),
    }


def emit_tile(nc, pools, x_src, y_dst, E):
    """Emit the full per-tile dataflow: x_src -> y_dst ([128, 3E] DRAM APs).

    Dot products are written in place into dead entry/product tiles (no
    m-pool), keeping the footprint at 60E B/partition so bufs=3 fits.
    Engine assignment found by greedy TimelineSim search."""
    xp, pp, ep = pools["xp"], pools["pp"], pools["ep"]
    X = xp.tile([128, 3 * E], FP32, tag="X", name="X")
    nc.sync.dma_start(X[:], x_src)
    u = X[:, 0:3 * E:3]
    x1 = X[:, 1:3 * E:3]
    x2 = X[:, 2:3 * E:3]

    v = pp.tile([128, E], FP32, tag="v", name="v")
    P = pp.tile([128, E], FP32, tag="P", name="P")
    Q = pp.tile([128, E], FP32, tag="Q", name="Q")
    # products
    nc.scalar.square(v[:], u)
    nc.vector.tensor_tensor(P[:], u, x1, MULT)
    nc.vector.tensor_tensor(Q[:], u, x2, MULT)

    # entry polynomials in v (dense affine ops)
    ah = ep.tile([128, E], FP32, tag="ah", name="ah")
    bh = ep.tile([128, E], FP32, tag="bh", name="bh")
    ch = ep.tile([128, E], FP32, tag="ch", name="ch")
    dh = ep.tile([128, E], FP32, tag="dh", name="dh")
    eh = ep.tile([128, E], FP32, tag="eh", name="eh")
    fh = ep.tile([128, E], FP32, tag="fh", name="fh")
    gh = ep.tile([128, E], FP32, tag="gh", name="gh")
    hh = ep.tile([128, E], FP32, tag="hh", name="hh")
    nc.scalar.activation(ah[:], v[:], IDENT, bias=A0, scale=A2)
    nc.scalar.activation(bh[:], v[:], IDENT, bias=B0, scale=B2)
    nc.scalar.activation(ch[:], v[:], IDENT, bias=C1, scale=C3)
    nc.scalar.activation(dh[:], v[:], IDENT, bias=D0, scale=D2)
    nc.scalar.activation(gh[:], v[:], IDENT, bias=G1, scale=G3)
    nc.vector.tensor_scalar(fh[:], v[:], F3, F1, MULT, ADD)
    nc.scalar.activation(eh[:], v[:], IDENT, bias=E0, scale=E2)
    nc.scalar.activation(hh[:], v[:], IDENT, bias=H0, scale=H2)

    # dot products, in place into dead tiles
    nc.vector.tensor_tensor(ah[:], ah[:], u, MULT)       # m1 -> ah
    nc.gpsimd.tensor_tensor(bh[:], bh[:], x1, MULT)      # m2 -> bh
    nc.gpsimd.tensor_tensor(ch[:], ch[:], Q[:], MULT)    # m3 -> ch
    nc.vector.tensor_tensor(dh[:], dh[:], u, MULT)       # m4 -> dh
    nc.vector.tensor_tensor(eh[:], eh[:], x1, MULT)      # m5 -> eh
    nc.vector.tensor_tensor(Q[:], fh[:], Q[:], MULT)     # m6 -> Q (after m3)
    nc.gpsimd.tensor_tensor(gh[:], gh[:], v[:], MULT)    # m7 -> gh
    nc.vector.tensor_tensor(hh[:], hh[:], x2, MULT)      # m8 -> hh
    nc.gpsimd.tensor_tensor(P[:], fh[:], P[:], MULT)     # m9 -> P

    # sums, then final strided writes back into X
    nc.vector.tensor_tensor(ah[:], ah[:], bh[:], ADD)
    nc.gpsimd.tensor_tensor(dh[:], dh[:], eh[:], ADD)
    nc.gpsimd.tensor_tensor(gh[:], gh[:], hh[:], ADD)
    nc.vector.tensor_tensor(X[:, 0:3 * E:3], ah[:], ch[:], ADD)
    nc.vector.tensor_tensor(X[:, 1:3 * E:3], dh[:], Q[:], ADD)
    nc.vector.tensor_tensor(X[:, 2:3 * E:3], gh[:], P[:], SUB)

    nc.sync.dma_start(y_dst, X[:])


def build_nc(E=E_DEF, T=T_DEF):
    """Build the per-core Bass program: x[T,128,3E] -> y[T,128,3E]."""
    nc = bass.Bass("TRN2", target_bir_lowering=False, debug=False)
    # register const APs for ACT-activation bias operands
    for val in (A0, B0, C1, D0, E0, G1, H0):
        t = nc.alloc_sbuf_tensor(f"const-f32-{val}", [128, 1], FP32)
        nc.gpsimd.memset(t.ap(), val)
        nc.const_aps.aps[(FP32, val)] = t.ap()
    nc.all_engine_barrier()
    x_d = nc.dram_tensor("x", [T, 128, 3 * E], FP32, kind="ExternalInput").ap()
    y_d = nc.dram_tensor("y", [T, 128, 3 * E], FP32, kind="ExternalOutput").ap()

    with tile.TileContext(nc) as tc, ExitStack() as ctx:
        pools = make_pools(tc, ctx)
        for t in range(T):
            emit_tile(nc, pools, x_d[t], y_d[t], E)

    _fix_tsp_waits(nc)
    return nc


def _fix_tsp_waits(nc):
    """Several TPB instruction encodings (S2S2D2_STT, pool S3S3D3_TT, ...)
    have a single sync-wait slot; Tile may attach several.  Hoist
    all-but-one wait onto same-engine nops inserted immediately before."""
    eng_map = {
        mybir.EngineType.DVE: nc.vector,
        mybir.EngineType.Activation: nc.scalar,
        mybir.EngineType.Pool: nc.gpsimd,
        mybir.EngineType.PE: nc.tensor,
        mybir.EngineType.SP: nc.sync,
    }
    for blk in nc.m.functions[0].blocks:
        i = 0
        while i < len(blk.instructions):
            ins = blk.instructions[i]
            if ins.sync_info:
                waits = list(ins.sync_info.on_wait)
                if len(waits) > 1:
                    extra, keep = waits[:-1], waits[-1:]
                    ins.sync_info.on_wait = keep
                    for w in extra:
                        eng_map[ins.engine].nop()
                        nop = nc.m.functions[0].blocks[-1].instructions.pop()
                        assert isinstance(nop, mybir.InstNoOp)
                        nop.sync_info = mybir.SyncInfo(on_wait=[w], on_update=[])
                        blk.instructions.insert(i, nop)
                        i += 1
            i += 1


_CACHE = {}


def _get_nc(E, T):
    key = (E, T)
    if key not in _CACHE:
        _CACHE[key] = build_nc(E, T)
    return _CACHE[key]


def kernel(x: np.ndarray) -> np.ndarray:
    E, T = E_DEF, T_DEF
    n_pc = 128 * E * T                  # elements per core
    b_pad = NCORES * n_pc
    B = x.shape[0]
    assert x.shape[1] == 3 and b_pad >= B

    nc = _get_nc(E, T)
    xp = np.zeros((b_pad, 3), dtype=np.float32)
    xp[:B] = x
    shards = xp.reshape(NCORES, T, 128, 3 * E)
    in_maps = [{"x": shards[c]} for c in range(NCORES)]
    res = run_bass_kernel_spmd(nc, in_maps, list(range(NCORES)))
    y = np.concatenate([r["y"].reshape(n_pc, 3) for r in res.results], axis=0)
    return y[:B]



# revision 2
# speedup vs baseline: 227.1701x; 227.1701x over previous
"""Trainium2 Bass kernel: batched 3x3 Lorenz-Jacobian Taylor matrix exponential.

Math (truncated Taylor expm, rel err ~1.9e-3 vs float64 reference):
    y0 = A0*u + B0*x1 + C1*(u*x2)
    y1 = D0*u + E0*x1 + F1*(u*x2)
    y2 = G1*u^2 + H0*x2 - F1*(u*x1)

Per tile ([128, 3E] fp16 in/out, dense slices):
  DVE : Q = u*x2; P = u*x1; z0 = B0*x1+t0; y0 = C1*Q+z0
  Pool: vq = u*u; issues output DMAs
  ACT : t0 = A0*u; y1 = copy(ps1); y2 = copy(ps2)   (copies pipelined)
  PE  : ps1 = (D0 I)@u + (E0 I)@x1 + (F1 I)@Q
        ps2 = (H0 I)@x2 + (G1 I)@vq + (-F1 I)@P
"""

import numpy as np
from contextlib import ExitStack

import concourse.bass as bass
import concourse.tile as tile
import concourse.mybir as mybir
from concourse.bass_utils import run_bass_kernel_spmd

A0 = 0.8679133685333335
B0 = 0.1827780802666667
C1 = -0.0018440311802469136
D0 = 0.5117786247466667
E0 = 1.0324136407733333
F1 = -0.019630097558847738
G1 = 0.005163287304691359
H0 = 0.9480639384616735

NCORES = 8
E_DEF = 489
T_DEF = 4
B_IN = 2_000_000

F16 = mybir.dt.float16
F32 = mybir.dt.float32
MULT = mybir.AluOpType.mult
ADD = mybir.AluOpType.add
COPY = mybir.ActivationFunctionType.Copy

W_COEF = ("D0", "E0", "F1", "H0", "G1", "mF1")


def build_nc(E=E_DEF, T=T_DEF):
    assert E <= 512
    nc = bass.Bass("TRN2", target_bir_lowering=False, debug=False)

    x_d = nc.dram_tensor("x", [T, 128, 3 * E], F16, kind="ExternalInput").ap()
    w_d = nc.dram_tensor("w", [128, 6 * 128], F16, kind="ExternalInput").ap()
    y_d = nc.dram_tensor("y", [T, 128, 3 * E], F16, kind="ExternalOutput").ap()

    with tile.TileContext(nc) as tc, ExitStack() as ctx:
        wp = ctx.enter_context(tc.tile_pool(name="wp", bufs=1))
        xp = ctx.enter_context(tc.tile_pool(name="xp", bufs=4))
        pp = ctx.enter_context(tc.tile_pool(name="pp", bufs=3))
        psp = ctx.enter_context(tc.psum_pool(name="psp", bufs=4))

        # tile 0's u-columns land first so compute can start ASAP
        Xs = [xp.tile([128, 3 * E], F16, tag="X", name=f"X{t}") for t in range(T)]
        Ys = [xp.tile([128, 3 * E], F16, tag="Y", name=f"Y{t}") for t in range(T)]
        nc.sync.dma_start(Xs[0][:, 0:E], x_d[0, :, 0:E])
        nc.sync.dma_start(Xs[0][:, E:3 * E], x_d[0, :, E:3 * E])
        W = wp.tile([128, 6 * 128], F16, tag="W", name="W")
        nc.sync.dma_start(W[:], w_d)
        for t in range(1, T):
            nc.sync.dma_start(Xs[t][:], x_d[t])
        wof = {nm: W[:, 128 * j:128 * (j + 1)] for j, nm in enumerate(W_COEF)}

        comp = [(Xs[t][:, 0:E], Xs[t][:, E:2 * E], Xs[t][:, 2 * E:3 * E])
                for t in range(T)]
        t0s, vqs, ps1s, ps2s = [], [], [], []
        for t in range(T):
            t0s.append(pp.tile([128, E], F16, tag="t0", name=f"t0_{t}"))
            vqs.append(pp.tile([128, E], F16, tag="vq", name=f"vq_{t}"))
            ps1s.append(psp.tile([128, E], F32, tag="ps1", name=f"ps1_{t}"))
            ps2s.append(psp.tile([128, E], F32, tag="ps2", name=f"ps2_{t}"))

        # ACT stream: t0(t) leads its tile; PSUM copies trail one tile behind
        def act_ops(t):
            u = comp[t][0]
            nc.scalar.activation(t0s[t][:], u, COPY, bias=0.0, scale=A0)

        def act_copies(t):
            nc.scalar.activation(Ys[t][:, E:2 * E], ps1s[t][:], COPY,
                                 bias=0.0, scale=1.0)
            nc.scalar.activation(Ys[t][:, 2 * E:3 * E], ps2s[t][:], COPY,
                                 bias=0.0, scale=1.0)

        # interleaved emission so no engine head-of-line-blocks on another
        pend_mms = []
        for t in range(T):
            u, x1, x2 = comp[t]
            Q = pp.tile([128, E], F16, tag="Q", name=f"Q{t}")
            P = pp.tile([128, E], F16, tag="P", name=f"P{t}")
            z0 = pp.tile([128, E], F16, tag="z0", name=f"z0_{t}")

            if t == 0:
                act_ops(0)
            nc.gpsimd.tensor_tensor(vqs[t][:], u, u, MULT)
            nc.vector.tensor_tensor(Q[:], u, x2, MULT)
            nc.vector.tensor_tensor(P[:], u, x1, MULT)
            if t + 1 < T:
                act_ops(t + 1)

            rhs_of = {"D0": u, "E0": x1, "F1": Q[:], "H0": x2,
                      "G1": vqs[t][:], "mF1": P[:]}
            pend_mms.append((t, rhs_of))

            nc.vector.scalar_tensor_tensor(z0[:], x1, B0, t0s[t][:], MULT, ADD)
            nc.vector.scalar_tensor_tensor(Ys[t][:, 0:E], Q[:], C1, z0[:], MULT, ADD)

            if len(pend_mms) == 2:
                for nm, pss, st, sp in (("D0", ps1s, True, False),
                                        ("E0", ps1s, False, False),
                                        ("F1", ps1s, False, True),
                                        ("H0", ps2s, True, False),
                                        ("G1", ps2s, False, False),
                                        ("mF1", ps2s, False, True)):
                    for tt, _rof in pend_mms:
                        nc.tensor.matmul(pss[tt][:], wof[nm], _rof[nm],
                                         start=st, stop=sp, skip_group_check=True)
                for tt, _rof in pend_mms:
                    act_copies(tt)
                    if tt == T - 1:
                        nc.sync.dma_start(y_d[tt, :, 0:2 * E], Ys[tt][:, 0:2 * E])
                        nc.sync.dma_start(y_d[tt, :, 2 * E:3 * E],
                                          Ys[tt][:, 2 * E:3 * E])
                    else:
                        nc.sync.dma_start(y_d[tt], Ys[tt][:])
                pend_mms.clear()

    _fix_tsp_waits(nc)
    return nc


def _fix_tsp_waits(nc):
    """Several TPB instruction encodings have a single sync-wait slot; Tile
    may attach several.  Hoist all-but-one onto same-engine nops."""
    eng_map = {
        mybir.EngineType.DVE: nc.vector,
        mybir.EngineType.Activation: nc.scalar,
        mybir.EngineType.Pool: nc.gpsimd,
        mybir.EngineType.PE: nc.tensor,
        mybir.EngineType.SP: nc.sync,
    }
    for blk in nc.m.functions[0].blocks:
        i = 0
        while i < len(blk.instructions):
            ins = blk.instructions[i]
            if ins.sync_info:
                waits = list(ins.sync_info.on_wait)
                if len(waits) > 1:
                    extra, keep = waits[:-1], waits[-1:]
                    ins.sync_info.on_wait = keep
                    for w in extra:
                        eng_map[ins.engine].nop()
                        nop = nc.m.functions[0].blocks[-1].instructions.pop()
                        assert isinstance(nop, mybir.InstNoOp)
                        nop.sync_info = mybir.SyncInfo(on_wait=[w], on_update=[])
                        blk.instructions.insert(i, nop)
                        i += 1
            i += 1


_CACHE = {}


def _get_nc(E=E_DEF, T=T_DEF):
    key = (E, T)
    if key not in _CACHE:
        _CACHE[key] = build_nc(E, T)
    return _CACHE[key]


def make_weights():
    w = np.zeros((128, 6 * 128), np.float16)
    idx = np.arange(128)
    for j, c in enumerate((D0, E0, F1, H0, G1, -F1)):
        w[idx, 128 * j + idx] = np.float16(c)
    return w


def prep_x(x, E=E_DEF, T=T_DEF):
    """[B,3] f32 -> [NCORES, T, 128, 3E] f16, components de-interleaved."""
    n_pc = 128 * E * T
    b_pad = NCORES * n_pc
    B = x.shape[0]
    xp = np.zeros((b_pad, 3), np.float16)
    xp[:B] = x.astype(np.float16)
    xr = (xp.reshape(NCORES, T, 128, E, 3)
            .transpose(0, 1, 2, 4, 3)
            .reshape(NCORES, T, 128, 3 * E))
    return np.ascontiguousarray(xr)


def unprep_y(ys, B, E=E_DEF, T=T_DEF):
    """list of per-core [T,128,3E] f16 -> [B,3] f32."""
    n_pc = 128 * E * T
    yr = (np.stack(ys, 0)
            .reshape(NCORES, T, 128, 3, E)
            .transpose(0, 1, 2, 4, 3)
            .reshape(NCORES * n_pc, 3))
    return np.ascontiguousarray(yr[:B]).astype(np.float32)


def kernel(x: np.ndarray) -> np.ndarray:
    E, T = E_DEF, T_DEF
    B = x.shape[0]
    assert x.shape[1] == 3 and NCORES * 128 * E * T >= B

    nc = _get_nc(E, T)
    shards = prep_x(x, E, T)
    w = make_weights()
    in_maps = [{"x": shards[c], "w": w} for c in range(NCORES)]
    res = run_bass_kernel_spmd(nc, in_maps, list(range(NCORES)))
    return unprep_y([r["y"] for r in res.results], B, E, T)


# revision 3
# speedup vs baseline: 265.9112x; 1.1705x over previous
"""Trainium2 Bass kernel: batched 3x3 Lorenz-Jacobian Taylor matrix exponential.

Math (truncated Taylor expm, rel err ~1.9e-3 vs float64 reference):
    y0 = A0*u + B0*x1 + C1*(u*x2)
    y1 = D0*u + E0*x1 + F1*(u*x2)
    y2 = G1*u^2 + H0*x2 - F1*(u*x1)

Per tile ([128, 3E] fp16 in/out, dense slices):
  DVE : Q = u*x2; P = u*x1; z0 = B0*x1+t0; y0 = C1*Q+z0
  Pool: vq = u*u; issues output DMAs
  ACT : t0 = A0*u; y1 = copy(ps1); y2 = copy(ps2)   (copies pipelined)
  PE  : ps1 = (D0 I)@u + (E0 I)@x1 + (F1 I)@Q
        ps2 = (H0 I)@x2 + (G1 I)@vq + (-F1 I)@P
"""

import numpy as np
from contextlib import ExitStack

import concourse.bass as bass
import concourse.tile as tile
import concourse.mybir as mybir
from concourse.bass_utils import run_bass_kernel_spmd

A0 = 0.8679133685333335
B0 = 0.1827780802666667
C1 = -0.0018440311802469136
D0 = 0.5117786247466667
E0 = 1.0324136407733333
F1 = -0.019630097558847738
G1 = 0.005163287304691359
H0 = 0.9480639384616735

NCORES = 8
E_DEF = 489
T_DEF = 4
B_IN = 2_000_000

F16 = mybir.dt.float16
F32 = mybir.dt.float32
MULT = mybir.AluOpType.mult
ADD = mybir.AluOpType.add
COPY = mybir.ActivationFunctionType.Copy

W_COEF = ("D0", "E0", "F1", "H0", "G1", "mF1")


def build_nc(E=E_DEF, T=T_DEF):
    assert E <= 512
    nc = bass.Bass("TRN2", target_bir_lowering=False, debug=False)

    x_d = nc.dram_tensor("x", [T, 128, 3 * E], F16, kind="ExternalInput").ap()
    w_d = nc.dram_tensor("w", [128, 6 * 128], F16, kind="ExternalInput").ap()
    y_d = nc.dram_tensor("y", [T, 128, 3 * E], F16, kind="ExternalOutput").ap()

    with tile.TileContext(nc) as tc, ExitStack() as ctx:
        wp = ctx.enter_context(tc.tile_pool(name="wp", bufs=1))
        xp = ctx.enter_context(tc.tile_pool(name="xp", bufs=4))
        pp = ctx.enter_context(tc.tile_pool(name="pp", bufs=3))
        psp = ctx.enter_context(tc.psum_pool(name="psp", bufs=4))

        # tile 0's u-columns land first so compute can start ASAP
        Xs = [xp.tile([128, 3 * E], F16, tag="X", name=f"X{t}") for t in range(T)]
        Ys = [xp.tile([128, 3 * E], F16, tag="Y", name=f"Y{t}") for t in range(T)]
        nc.sync.dma_start(Xs[0][:, 0:E], x_d[0, :, 0:E])
        nc.sync.dma_start(Xs[0][:, E:3 * E], x_d[0, :, E:3 * E])
        nc.sync.dma_start(Xs[1][:, 0:E], x_d[1, :, 0:E])
        nc.sync.dma_start(Xs[1][:, E:3 * E], x_d[1, :, E:3 * E])
        W = wp.tile([128, 6 * 128], F16, tag="W", name="W")
        nc.sync.dma_start(W[:], w_d)
        for t in range(2, T):
            nc.sync.dma_start(Xs[t][:], x_d[t])
        wof = {nm: W[:, 128 * j:128 * (j + 1)] for j, nm in enumerate(W_COEF)}

        comp = [(Xs[t][:, 0:E], Xs[t][:, E:2 * E], Xs[t][:, 2 * E:3 * E])
                for t in range(T)]
        t0s, vqs, ps1s, ps2s = [], [], [], []
        for t in range(T):
            t0s.append(pp.tile([128, E], F16, tag="t0", name=f"t0_{t}"))
            vqs.append(pp.tile([128, E], F16, tag="vq", name=f"vq_{t}"))
            ps1s.append(psp.tile([128, E], F32, tag="ps1", name=f"ps1_{t}"))
            ps2s.append(psp.tile([128, E], F32, tag="ps2", name=f"ps2_{t}"))

        # ACT stream: t0(t) leads its tile; PSUM copies trail one tile behind
        def act_ops(t):
            u = comp[t][0]
            nc.scalar.activation(t0s[t][:], u, COPY, bias=0.0, scale=A0)

        def act_copies(t):
            nc.scalar.activation(Ys[t][:, E:2 * E], ps1s[t][:], COPY,
                                 bias=0.0, scale=1.0)
            nc.scalar.activation(Ys[t][:, 2 * E:3 * E], ps2s[t][:], COPY,
                                 bias=0.0, scale=1.0)

        # interleaved emission so no engine head-of-line-blocks on another
        pend_mms = []
        for t in range(T):
            u, x1, x2 = comp[t]
            Q = pp.tile([128, E], F16, tag="Q", name=f"Q{t}")
            P = pp.tile([128, E], F16, tag="P", name=f"P{t}")
            z0 = pp.tile([128, E], F16, tag="z0", name=f"z0_{t}")

            if t == 0:
                act_ops(0)
            nc.gpsimd.tensor_tensor(vqs[t][:], u, u, MULT)
            nc.vector.tensor_tensor(Q[:], u, x2, MULT)
            nc.vector.tensor_tensor(P[:], u, x1, MULT)
            if t + 1 < T:
                act_ops(t + 1)

            rhs_of = {"D0": u, "E0": x1, "F1": Q[:], "H0": x2,
                      "G1": vqs[t][:], "mF1": P[:]}
            pend_mms.append((t, rhs_of))

            nc.vector.scalar_tensor_tensor(z0[:], x1, B0, t0s[t][:], MULT, ADD)
            nc.vector.scalar_tensor_tensor(Ys[t][:, 0:E], Q[:], C1, z0[:], MULT, ADD)

            if len(pend_mms) == 2:
                for nm, pss, st, sp in (("D0", ps1s, True, False),
                                        ("E0", ps1s, False, False),
                                        ("F1", ps1s, False, True),
                                        ("H0", ps2s, True, False),
                                        ("G1", ps2s, False, False),
                                        ("mF1", ps2s, False, True)):
                    for tt, _rof in pend_mms:
                        nc.tensor.matmul(pss[tt][:], wof[nm], _rof[nm],
                                         start=st, stop=sp, skip_group_check=True)
                for tt, _rof in pend_mms:
                    act_copies(tt)
                    if tt == T - 1:
                        nc.sync.dma_start(y_d[tt, :, 0:2 * E], Ys[tt][:, 0:2 * E])
                        nc.sync.dma_start(y_d[tt, :, 2 * E:3 * E],
                                          Ys[tt][:, 2 * E:3 * E])
                    else:
                        nc.sync.dma_start(y_d[tt], Ys[tt][:])
                pend_mms.clear()

    _fix_tsp_waits(nc)
    return nc


def _fix_tsp_waits(nc):
    """Several TPB instruction encodings have a single sync-wait slot; Tile
    may attach several.  Hoist all-but-one onto same-engine nops."""
    eng_map = {
        mybir.EngineType.DVE: nc.vector,
        mybir.EngineType.Activation: nc.scalar,
        mybir.EngineType.Pool: nc.gpsimd,
        mybir.EngineType.PE: nc.tensor,
        mybir.EngineType.SP: nc.sync,
    }
    for blk in nc.m.functions[0].blocks:
        i = 0
        while i < len(blk.instructions):
            ins = blk.instructions[i]
            if ins.sync_info:
                waits = list(ins.sync_info.on_wait)
                if len(waits) > 1:
                    extra, keep = waits[:-1], waits[-1:]
                    ins.sync_info.on_wait = keep
                    for w in extra:
                        eng_map[ins.engine].nop()
                        nop = nc.m.functions[0].blocks[-1].instructions.pop()
                        assert isinstance(nop, mybir.InstNoOp)
                        nop.sync_info = mybir.SyncInfo(on_wait=[w], on_update=[])
                        blk.instructions.insert(i, nop)
                        i += 1
            i += 1


_CACHE = {}


def _get_nc(E=E_DEF, T=T_DEF):
    key = (E, T)
    if key not in _CACHE:
        _CACHE[key] = build_nc(E, T)
    return _CACHE[key]


def make_weights():
    w = np.zeros((128, 6 * 128), np.float16)
    idx = np.arange(128)
    for j, c in enumerate((D0, E0, F1, H0, G1, -F1)):
        w[idx, 128 * j + idx] = np.float16(c)
    return w


def prep_x(x, E=E_DEF, T=T_DEF):
    """[B,3] f32 -> [NCORES, T, 128, 3E] f16, components de-interleaved."""
    n_pc = 128 * E * T
    b_pad = NCORES * n_pc
    B = x.shape[0]
    xp = np.zeros((b_pad, 3), np.float16)
    xp[:B] = x.astype(np.float16)
    xr = (xp.reshape(NCORES, T, 128, E, 3)
            .transpose(0, 1, 2, 4, 3)
            .reshape(NCORES, T, 128, 3 * E))
    return np.ascontiguousarray(xr)


def unprep_y(ys, B, E=E_DEF, T=T_DEF):
    """list of per-core [T,128,3E] f16 -> [B,3] f32."""
    n_pc = 128 * E * T
    yr = (np.stack(ys, 0)
            .reshape(NCORES, T, 128, 3, E)
            .transpose(0, 1, 2, 4, 3)
            .reshape(NCORES * n_pc, 3))
    return np.ascontiguousarray(yr[:B]).astype(np.float32)


def kernel(x: np.ndarray) -> np.ndarray:
    E, T = E_DEF, T_DEF
    B = x.shape[0]
    assert x.shape[1] == 3 and NCORES * 128 * E * T >= B

    nc = _get_nc(E, T)
    shards = prep_x(x, E, T)
    w = make_weights()
    in_maps = [{"x": shards[c], "w": w} for c in range(NCORES)]
    res = run_bass_kernel_spmd(nc, in_maps, list(range(NCORES)))
    return unprep_y([r["y"] for r in res.results], B, E, T)
